# revision 20
# baseline (speedup 1.0000x reference)
"""Multi-head self-attention (B=2, N=2048, C=1024, H=16, D=64) on 8 trn2 cores.

Sharding: core c handles batch b = c//4 and the 4 heads [4*(c%4), 4*(c%4)+4).
Host pre-transposes x and the weight slices (and pre-rounds them to the fp32r
grid) so the device kernel needs no on-chip transposes; per-core partial
outputs are summed on the host and the output bias is added there.

Device kernel (per core, matmuls in fp32r = TF32-like single-pass PE):
  phase 1: qkT = w_qkT.T @ xT  -> [512, 2048] SBUF ; V = x @ w_v.T (+ ones col)
  phase 2: per head pair, per 512-query tile: S.T = kT.T @ qT (row-tiled D=64
           pairs), exp on ScalarE from PSUM (scale=1/8), O.T = [V|1].T @ P.T
           accumulated over j; PSUM row 64 = softmax denominator r; Õ/r evicted
           fast, 1/r via approx reciprocal + rank-1 PE broadcast, DVE multiply.
  phase 3 (interleaved per i-tile): y = O.T.T @ woutT, DMA out.
"""

import numpy as np

import concourse.bass as bass
import concourse.mybir as mybir
import concourse.tile as tile
from concourse import bacc
from concourse.bass_utils import run_bass_kernel_spmd

F32 = mybir.dt.float32
F32R = mybir.dt.float32r   # TF32-like: 11 mantissa bits, 4x faster PE streaming

B, N, C = 2, 2048, 1024
H, D = 16, 64
HPC = 4            # heads per core
P = 128
FD = 512           # matmul free-dim tile
KB = C // P        # 8 contraction blocks for the projections
NT = N // FD       # 4 free tiles over the sequence
IT = N // P        # 16 row blocks of 128
NJB = N // P       # 16 j blocks in attention



def _attn_jbs(nc, ps, pt_pool, qkT_sb, V_sb, ot, pr, itl, jbs,
              do_st, do_exp, do_pv):
    for jb in jbs:
        st = ps.tile([P, 2 * FD], F32, tag="st", bufs=2, name="st")
        pt = pt_pool.tile([P, 2 * FD], F32R, tag="pt", name="pt")
        if do_st:
            for hh in range(2):
                lo = hh * D
                nc.tensor.matmul(
                    st[:, hh * FD:(hh + 1) * FD],
                    lhsT=qkT_sb[lo:lo + D, 2 + pr, jb * P:(jb + 1) * P],
                    rhs=qkT_sb[lo:lo + D, pr, itl * FD:(itl + 1) * FD],
                    start=True, stop=True)
        if do_exp:
            nc.scalar.activation(
                pt, st, mybir.ActivationFunctionType.Exp, scale=0.125)
        if do_pv:
            for hh in range(2):
                nc.tensor.matmul(
                    ot[hh],
                    lhsT=V_sb[:, jb, 2 * pr + hh, :],
                    rhs=pt[:, hh * FD:(hh + 1) * FD],
                    start=(jb == 0), stop=(jb == NJB - 1))


def _attn_norm(nc, ps, nrm_pool, oT_sb, ones_sb, ot, pr, itl, MMB):
    for hh in range(2):
        osl = oT_sb[:, 2 * pr + hh, itl * FD:(itl + 1) * FD]
        nc.vector.tensor_copy(osl, ot[hh][0:D, :])
        rin = nrm_pool.tile([1, FD], F32, tag="rin", name="rin")
        nc.vector.tensor_copy(rin, ot[hh][D:D + 1, :])
        rec = nrm_pool.tile([1, FD], F32, tag="rec", name="rec")
        scr = nrm_pool.tile([1, FD], F32, tag="scr", name="scr")
        nc.vector.reciprocal_approx_accurate(out=rec, in_=rin, scratch=scr)
        rec2 = nrm_pool.tile([1, FD], F32R, tag="rec2", name="rec2")
        nc.vector.tensor_copy(rec2, rec)
        rep = ps.tile([D, FD], F32, tag="mm", bufs=MMB, name="rep")
        nc.tensor.matmul(rep, lhsT=ones_sb, rhs=rec2, start=True, stop=True)
        nc.vector.tensor_mul(out=osl, in0=osl, in1=rep)


def build_nc(repeat: int = 1, do_p1=True, do_p2=True, do_p3=True,
             do_st=True, do_exp=True, do_pv=True, do_nrm=True,
             OTB=2, MMB=2) -> bacc.Bacc:
    nc = bacc.Bacc("TRN2", target_bir_lowering=False, debug=False)

    xT = nc.dram_tensor("xT", [C, N], F32, kind="ExternalInput").ap()
    wqkvT = nc.dram_tensor("wqkvT", [C, 3 * HPC * D], F32, kind="ExternalInput").ap()
    woutT = nc.dram_tensor("woutT", [HPC, D, C], F32, kind="ExternalInput").ap()
    vones = nc.dram_tensor("vones", [P, NJB * HPC], F32,
                           kind="ExternalInput").ap()
    y = nc.dram_tensor("y", [N, C], F32, kind="ExternalOutput").ap()

    xT_r = xT.rearrange("(o p) n -> p o n", p=P)          # [128, 8, 2048]
    wqkvT_r = wqkvT.rearrange("(o p) f -> p o f", p=P)    # [128, 8, 768]
    woutT_r = woutT.rearrange("g p o -> p g o")           # [64, 4, 1024]

    with tile.TileContext(nc) as tc:
        with (
            tc.tile_pool(name="w_pool", bufs=1) as w_pool,
            tc.tile_pool(name="qk_pool", bufs=1) as qk_pool,
            tc.tile_pool(name="v_pool", bufs=1) as v_pool,
            tc.tile_pool(name="o_pool", bufs=1) as o_pool,
            tc.tile_pool(name="x_pool", bufs=2) as x_pool,
            tc.tile_pool(name="pt_pool", bufs=5) as pt_pool,
            tc.tile_pool(name="y_pool", bufs=3) as y_pool,
            tc.tile_pool(name="nrm_pool", bufs=2) as nrm_pool,
            tc.tile_pool(name="ps", bufs=1, space="PSUM") as ps,
        ):
            wq_sb = w_pool.tile([P, KB, 3 * HPC * D], F32R)
            nc.sync.dma_start(wq_sb[:, 0:4, :], wqkvT_r[:, 0:4, :].bitcast(F32R))
            nc.scalar.dma_start(wq_sb[:, 4:8, :], wqkvT_r[:, 4:8, :].bitcast(F32R))
            wo_sb = w_pool.tile([D, HPC, C], F32R)
            nc.scalar.dma_start(wo_sb, woutT_r.bitcast(F32R))
            ones_sb = w_pool.tile([1, D], F32R)
            nc.sync.dma_start(ones_sb, vones[0:1, 0:D].bitcast(F32R))

            qkT_sb = qk_pool.tile([P, 4, N], F32R)       # q01 | q23 | k01 | k23
            V_sb = v_pool.tile([P, NJB, HPC, D + 1], F32R)
            oT_sb = o_pool.tile([D, HPC, N], F32R)
            nc.gpsimd.dma_start(
                V_sb[:, :, :, D:D + 1].rearrange("p j h one -> p j (h one)"),
                vones.rearrange("p (j h) -> p j h", h=HPC).bitcast(F32R),
            )

            for _rep in range(repeat):
                # ---------------- phase 1: projections ----------------
                # The first attention block (pr0, itl0) is interleaved into
                # the chunk loop: its jb range [4nt, 4nt+4) only needs the
                # chunk just produced, so ScalarE starts exp'ing early.
                hoist = do_p1 and do_p2
                if hoist:
                    ot0 = [ps.tile([D + 1, FD], F32, tag="ot", bufs=OTB,
                                   name=f"ot0{hh}") for hh in range(2)]
                if do_p1:
                    for nt in range(NT):
                        xc = x_pool.tile([P, KB, FD], F32R, tag="xc")
                        nc.sync.dma_start(
                            xc[:, 0:KB // 2, :],
                            xT_r[:, 0:KB // 2, nt * FD:(nt + 1) * FD].bitcast(F32R))
                        nc.gpsimd.dma_start(
                            xc[:, KB // 2:KB, :],
                            xT_r[:, KB // 2:KB, nt * FD:(nt + 1) * FD].bitcast(F32R))
                        for mt in (2, 0, 1, 3) if nt == 0 else range(4):
                            pq = ps.tile([P, FD], F32, tag="mm", bufs=MMB,
                                         name="pq")
                            for kb in range(KB):
                                nc.tensor.matmul(
                                    pq,
                                    lhsT=wq_sb[:, kb, mt * P:(mt + 1) * P],
                                    rhs=xc[:, kb, :],
                                    start=(kb == 0),
                                    stop=(kb == KB - 1),
                                )
                            nc.vector.tensor_copy(
                                qkT_sb[:, mt, nt * FD:(nt + 1) * FD], pq)
                        for i4 in range(4):
                            it = nt * 4 + i4
                            pv = ps.tile([P, HPC * D], F32, tag="mm", bufs=MMB,
                                         name="pv")
                            for kb in range(KB):
                                nc.tensor.matmul(
                                    pv,
                                    lhsT=xc[:, kb, i4 * P:(i4 + 1) * P],
                                    rhs=wq_sb[:, kb, 2 * HPC * D:3 * HPC * D],
                                    start=(kb == 0),
                                    stop=(kb == KB - 1),
                                )
                            nc.vector.tensor_copy(
                                V_sb[:, it, :, 0:D],
                                pv.rearrange("p (h d) -> p h d", d=D),
                            )
                        if hoist:
                            _attn_jbs(nc, ps, pt_pool, qkT_sb, V_sb, ot0,
                                      0, 0, range(nt * 4, nt * 4 + 4),
                                      do_st, do_exp, do_pv)
                if hoist and do_nrm and do_pv:
                    _attn_norm(nc, ps, nrm_pool, oT_sb, ones_sb, ot0, 0, 0,
                               MMB)

                # -------- phase 2 + 3 interleaved per i tile --------
                for itl in range(NT):
                    if do_p2:
                        for pr in range(2):      # head pair (2*pr, 2*pr+1)
                            if hoist and pr == 0 and itl == 0:
                                continue
                            ot = [ps.tile([D + 1, FD], F32, tag="ot",
                                          bufs=OTB, name=f"ot{hh}")
                                  for hh in range(2)]
                            _attn_jbs(nc, ps, pt_pool, qkT_sb, V_sb, ot,
                                      pr, itl, range(NJB),
                                      do_st, do_exp, do_pv)
                            if do_nrm and do_pv:
                                _attn_norm(nc, ps, nrm_pool, oT_sb, ones_sb,
                                           ot, pr, itl, MMB)
                    # ------ phase 3 for the 4 row-blocks of this i tile ------
                    if do_p3:
                        for i4 in range(4):
                            it = itl * 4 + i4
                            y_t = y_pool.tile([P, C], F32, tag="yt")
                            for o2 in range(2):
                                py = ps.tile([P, FD], F32, tag="mm", bufs=MMB,
                                             name="py")
                                for g in range(HPC):
                                    nc.tensor.matmul(
                                        py,
                                        lhsT=oT_sb[:, g, it * P:(it + 1) * P],
                                        rhs=wo_sb[:, g, o2 * FD:(o2 + 1) * FD],
                                        start=(g == 0),
                                        stop=(g == HPC - 1),
                                    )
                                nc.vector.tensor_copy(
                                    y_t[:, o2 * FD:(o2 + 1) * FD], py)
                            eng = nc.sync if it % 2 == 0 else nc.gpsimd
                            eng.dma_start(y[it * P:(it + 1) * P, :], y_t)

    nc.finalize()
    return nc


def round_f32r(a):
    """Round fp32 array to the fp32r grid (11 mantissa bits, RNE)."""
    u = np.ascontiguousarray(a, dtype=np.float32).view(np.uint32)
    u = (u + 0x7FF + ((u >> 12) & 1)) & np.uint32(0xFFFFF000)
    return u.view(np.float32)


def shard_inputs(x, w_qkv, w_out):
    """Full inputs -> list of 8 per-core input maps (host-side prep)."""
    x = np.asarray(x, dtype=np.float32)
    w_qkv = np.asarray(w_qkv, dtype=np.float32)
    w_out = np.asarray(w_out, dtype=np.float32)
    in_maps = []
    for c in range(8):
        b, hp = c // 4, c % 4
        rows = np.concatenate(
            [w_qkv[q * C + hp * HPC * D:(q * C + (hp + 1) * HPC * D)]
             for q in range(3)], axis=0)                      # [768, C]
        in_maps.append({
            "vones": np.ones((P, NJB * HPC), np.float32),
            "xT": round_f32r(x[b].T),                          # [C, N]
            "wqkvT": round_f32r(rows.T),                       # [C, 768]
            "woutT": round_f32r(
                w_out[:, hp * HPC * D:(hp + 1) * HPC * D].T   # [256, C]
                .reshape(HPC, D, C)),
        })
    return in_maps


def combine_outputs(ys, b_out):
    b_out = np.asarray(b_out, dtype=np.float32)
    out0 = ys[0] + ys[1] + ys[2] + ys[3]
    out1 = ys[4] + ys[5] + ys[6] + ys[7]
    return np.stack([out0, out1], axis=0) + b_out[None, None, :]


_NC = None


def kernel(x, w_qkv, w_out, b_out):
    global _NC
    if _NC is None:
        _NC = build_nc()
    in_maps = shard_inputs(x, w_qkv, w_out)
    res = run_bass_kernel_spmd(_NC, in_maps, core_ids=list(range(8)))
    ys = [res.results[c]["y"] for c in range(8)]
    return combine_outputs(ys, b_out).astype(np.float32)


# revision 21
# speedup vs baseline: 1.0689x; 1.0689x over previous
"""Multi-head self-attention (B=2, N=2048, C=1024, H=16, D=64) on 8 trn2 cores.

Sharding: core c handles batch b = c//4 and the 4 heads [4*(c%4), 4*(c%4)+4).
Host pre-transposes x and the weight slices (and pre-rounds them to the fp32r
grid) so the device kernel needs no on-chip transposes; per-core partial
outputs are summed on the host and the output bias is added there.

Device kernel (per core, matmuls in fp32r = TF32-like single-pass PE):
  phase 1: qkT = w_qkT.T @ xT  -> [512, 2048] SBUF ; V = x @ w_v.T (+ ones col)
  phase 2: per head pair, per 512-query tile: S.T = kT.T @ qT (row-tiled D=64
           pairs), exp on ScalarE from PSUM (scale=1/8), O.T = [V|1].T @ P.T
           accumulated over j; PSUM row 64 = softmax denominator r; Õ/r evicted
           fast, 1/r via approx reciprocal + rank-1 PE broadcast, DVE multiply.
  phase 3 (interleaved per i-tile): y = O.T.T @ woutT, DMA out.
"""

import numpy as np

import concourse.bass as bass
import concourse.mybir as mybir
import concourse.tile as tile
from concourse import bacc
from concourse.bass_utils import run_bass_kernel_spmd

F32 = mybir.dt.float32
F32R = mybir.dt.float32r   # TF32-like: 11 mantissa bits, 4x faster PE streaming

B, N, C = 2, 2048, 1024
H, D = 16, 64
HPC = 4            # heads per core
P = 128
FD = 512           # matmul free-dim tile
KB = C // P        # 8 contraction blocks for the projections
NT = N // FD       # 4 free tiles over the sequence
IT = N // P        # 16 row blocks of 128
NJB = N // P       # 16 j blocks in attention



def _attn_jbs(nc, ps, pt_pool, qkT_sb, V_sb, ot, pr, itl, jbs,
              do_st, do_exp, do_pv):
    for jb in jbs:
        st = ps.tile([P, 2 * FD], F32, tag="st", bufs=2, name="st")
        pt = pt_pool.tile([P, 2 * FD], F32R, tag="pt", name="pt")
        if do_st:
            for hh in range(2):
                lo = hh * D
                nc.tensor.matmul(
                    st[:, hh * FD:(hh + 1) * FD],
                    lhsT=qkT_sb[lo:lo + D, 2 + pr, jb * P:(jb + 1) * P],
                    rhs=qkT_sb[lo:lo + D, pr, itl * FD:(itl + 1) * FD],
                    start=True, stop=True)
        if do_exp:
            nc.scalar.activation(
                pt, st, mybir.ActivationFunctionType.Exp, scale=0.125)
        if do_pv:
            for hh in range(2):
                nc.tensor.matmul(
                    ot[hh],
                    lhsT=V_sb[:, jb, 2 * pr + hh, :],
                    rhs=pt[:, hh * FD:(hh + 1) * FD],
                    start=(jb == 0), stop=(jb == NJB - 1))


def _attn_norm(nc, ps, nrm_pool, oT_sb, ones_sb, ot, pr, itl, MMB):
    for hh in range(2):
        osl = oT_sb[:, 2 * pr + hh, itl * FD:(itl + 1) * FD]
        nc.vector.tensor_copy(osl, ot[hh][0:D, :])
        rin = nrm_pool.tile([1, FD], F32, tag="rin", name="rin")
        nc.vector.tensor_copy(rin, ot[hh][D:D + 1, :])
        rec = nrm_pool.tile([1, FD], F32, tag="rec", name="rec")
        scr = nrm_pool.tile([1, FD], F32, tag="scr", name="scr")
        nc.vector.reciprocal_approx_accurate(out=rec, in_=rin, scratch=scr)
        rec2 = nrm_pool.tile([1, FD], F32R, tag="rec2", name="rec2")
        nc.vector.tensor_copy(rec2, rec)
        rep = ps.tile([D, FD], F32, tag="mm", bufs=MMB, name="rep")
        nc.tensor.matmul(rep, lhsT=ones_sb, rhs=rec2, start=True, stop=True)
        nc.vector.tensor_mul(out=osl, in0=osl, in1=rep)


def build_nc(repeat: int = 1, do_p1=True, do_p2=True, do_p3=True,
             do_st=True, do_exp=True, do_pv=True, do_nrm=True,
             OTB=2, MMB=2) -> bacc.Bacc:
    nc = bacc.Bacc("TRN2", target_bir_lowering=False, debug=False)

    xT = nc.dram_tensor("xT", [C, N], F32, kind="ExternalInput").ap()
    wqkvT = nc.dram_tensor("wqkvT", [C, 3 * HPC * D], F32, kind="ExternalInput").ap()
    woutT = nc.dram_tensor("woutT", [HPC, D, C], F32, kind="ExternalInput").ap()
    vones = nc.dram_tensor("vones", [P, NJB * HPC], F32,
                           kind="ExternalInput").ap()
    y = nc.dram_tensor("y", [N, C], F32, kind="ExternalOutput").ap()

    xT_r = xT.rearrange("(o p) n -> p o n", p=P)          # [128, 8, 2048]
    wqkvT_r = wqkvT.rearrange("(o p) f -> p o f", p=P)    # [128, 8, 768]
    woutT_r = woutT.rearrange("g p o -> p g o")           # [64, 4, 1024]

    with tile.TileContext(nc) as tc:
        with (
            tc.tile_pool(name="w_pool", bufs=1) as w_pool,
            tc.tile_pool(name="qk_pool", bufs=1) as qk_pool,
            tc.tile_pool(name="v_pool", bufs=1) as v_pool,
            tc.tile_pool(name="o_pool", bufs=1) as o_pool,
            tc.tile_pool(name="x_pool", bufs=2) as x_pool,
            tc.tile_pool(name="pt_pool", bufs=4) as pt_pool,
            tc.tile_pool(name="y_pool", bufs=3) as y_pool,
            tc.tile_pool(name="nrm_pool", bufs=2) as nrm_pool,
            tc.tile_pool(name="ps", bufs=1, space="PSUM") as ps,
        ):
            wq_sb = w_pool.tile([P, KB, 3 * HPC * D], F32R)
            nc.sync.dma_start(wq_sb[:, 0:4, :], wqkvT_r[:, 0:4, :].bitcast(F32R))
            nc.scalar.dma_start(wq_sb[:, 4:8, :], wqkvT_r[:, 4:8, :].bitcast(F32R))
            wo_sb = w_pool.tile([D, HPC, C], F32R)
            nc.scalar.dma_start(wo_sb, woutT_r.bitcast(F32R))
            ones_sb = w_pool.tile([1, D], F32R)
            nc.sync.dma_start(ones_sb, vones[0:1, 0:D].bitcast(F32R))

            qkT_sb = qk_pool.tile([P, 4, N], F32R)       # q01 | q23 | k01 | k23
            V_sb = v_pool.tile([P, NJB, HPC, D + 1], F32R)
            oT_sb = o_pool.tile([D, HPC, N], F32R)
            nc.gpsimd.dma_start(
                V_sb[:, :, :, D:D + 1].rearrange("p j h one -> p j (h one)"),
                vones.rearrange("p (j h) -> p j h", h=HPC).bitcast(F32R),
            )

            for _rep in range(repeat):
                # ---------------- phase 1: projections ----------------
                # The first attention block (pr0, itl0) is interleaved into
                # the chunk loop: its jb range [4nt, 4nt+4) only needs the
                # chunk just produced, so ScalarE starts exp'ing early.
                hoist = do_p1 and do_p2
                if hoist:
                    ot0 = [ps.tile([D + 1, FD], F32, tag="ot", bufs=OTB,
                                   name=f"ot0{hh}") for hh in range(2)]
                if do_p1:
                    for nt in range(NT):
                        xc = x_pool.tile([P, KB, FD], F32R, tag="xc")
                        nc.sync.dma_start(
                            xc[:, 0:KB // 2, :],
                            xT_r[:, 0:KB // 2, nt * FD:(nt + 1) * FD].bitcast(F32R))
                        nc.gpsimd.dma_start(
                            xc[:, KB // 2:KB, :],
                            xT_r[:, KB // 2:KB, nt * FD:(nt + 1) * FD].bitcast(F32R))
                        for mt in (2, 0, 1, 3) if nt == 0 else range(4):
                            pq = ps.tile([P, FD], F32, tag="mm", bufs=MMB,
                                         name="pq")
                            for kb in range(KB):
                                nc.tensor.matmul(
                                    pq,
                                    lhsT=wq_sb[:, kb, mt * P:(mt + 1) * P],
                                    rhs=xc[:, kb, :],
                                    start=(kb == 0),
                                    stop=(kb == KB - 1),
                                )
                            nc.vector.tensor_copy(
                                qkT_sb[:, mt, nt * FD:(nt + 1) * FD], pq)
                        for i4 in range(4):
                            it = nt * 4 + i4
                            pv = ps.tile([P, HPC * D], F32, tag="mm", bufs=MMB,
                                         name="pv")
                            for kb in range(KB):
                                nc.tensor.matmul(
                                    pv,
                                    lhsT=xc[:, kb, i4 * P:(i4 + 1) * P],
                                    rhs=wq_sb[:, kb, 2 * HPC * D:3 * HPC * D],
                                    start=(kb == 0),
                                    stop=(kb == KB - 1),
                                )
                            nc.vector.tensor_copy(
                                V_sb[:, it, :, 0:D],
                                pv.rearrange("p (h d) -> p h d", d=D),
                            )
                        if hoist:
                            _attn_jbs(nc, ps, pt_pool, qkT_sb, V_sb, ot0,
                                      0, 0, range(nt * 4, nt * 4 + 4),
                                      do_st, do_exp, do_pv)
                if hoist and do_nrm and do_pv:
                    _attn_norm(nc, ps, nrm_pool, oT_sb, ones_sb, ot0, 0, 0,
                               MMB)

                # -------- phase 2 + 3 interleaved per i tile --------
                for itl in range(NT):
                    if do_p2:
                        for pr in range(2):      # head pair (2*pr, 2*pr+1)
                            if hoist and pr == 0 and itl == 0:
                                continue
                            ot = [ps.tile([D + 1, FD], F32, tag="ot",
                                          bufs=OTB, name=f"ot{hh}")
                                  for hh in range(2)]
                            _attn_jbs(nc, ps, pt_pool, qkT_sb, V_sb, ot,
                                      pr, itl, range(NJB),
                                      do_st, do_exp, do_pv)
                            if do_nrm and do_pv:
                                _attn_norm(nc, ps, nrm_pool, oT_sb, ones_sb,
                                           ot, pr, itl, MMB)
                    # ------ phase 3 for the 4 row-blocks of this i tile ------
                    if do_p3:
                        for i4 in range(4):
                            it = itl * 4 + i4
                            y_t = y_pool.tile([P, C], F32, tag="yt")
                            for o2 in range(2):
                                py = ps.tile([P, FD], F32, tag="mm", bufs=MMB,
                                             name="py")
                                for g in range(HPC):
                                    nc.tensor.matmul(
                                        py,
                                        lhsT=oT_sb[:, g, it * P:(it + 1) * P],
                                        rhs=wo_sb[:, g, o2 * FD:(o2 + 1) * FD],
                                        start=(g == 0),
                                        stop=(g == HPC - 1),
                                    )
                                nc.vector.tensor_copy(
                                    y_t[:, o2 * FD:(o2 + 1) * FD], py)
                            eng = nc.sync if it % 2 == 0 else nc.gpsimd
                            eng.dma_start(y[it * P:(it + 1) * P, :], y_t)

    nc.finalize()
    return nc


def round_f32r(a):
    """Round fp32 array to the fp32r grid (11 mantissa bits, RNE)."""
    u = np.ascontiguousarray(a, dtype=np.float32).view(np.uint32)
    u = (u + 0x7FF + ((u >> 12) & 1)) & np.uint32(0xFFFFF000)
    return u.view(np.float32)


def shard_inputs(x, w_qkv, w_out):
    """Full inputs -> list of 8 per-core input maps (host-side prep)."""
    x = np.asarray(x, dtype=np.float32)
    w_qkv = np.asarray(w_qkv, dtype=np.float32)
    w_out = np.asarray(w_out, dtype=np.float32)
    in_maps = []
    for c in range(8):
        b, hp = c // 4, c % 4
        rows = np.concatenate(
            [w_qkv[q * C + hp * HPC * D:(q * C + (hp + 1) * HPC * D)]
             for q in range(3)], axis=0)                      # [768, C]
        in_maps.append({
            "vones": np.ones((P, NJB * HPC), np.float32),
            "xT": round_f32r(x[b].T),                          # [C, N]
            "wqkvT": round_f32r(rows.T),                       # [C, 768]
            "woutT": round_f32r(
                w_out[:, hp * HPC * D:(hp + 1) * HPC * D].T   # [256, C]
                .reshape(HPC, D, C)),
        })
    return in_maps


def combine_outputs(ys, b_out):
    b_out = np.asarray(b_out, dtype=np.float32)
    out0 = ys[0] + ys[1] + ys[2] + ys[3]
    out1 = ys[4] + ys[5] + ys[6] + ys[7]
    return np.stack([out0, out1], axis=0) + b_out[None, None, :]


_NC = None


def kernel(x, w_qkv, w_out, b_out):
    global _NC
    if _NC is None:
        _NC = build_nc()
    in_maps = shard_inputs(x, w_qkv, w_out)
    res = run_bass_kernel_spmd(_NC, in_maps, core_ids=list(range(8)))
    ys = [res.results[c]["y"] for c in range(8)]
    return combine_outputs(ys, b_out).astype(np.float32)
